# revision 8
# baseline (speedup 1.0000x reference)
"""Trainium2 Bass kernel for the BioHAN model (8-core data-parallel SPMD).

Strategy:
  - Shard the batch dim (B=2048) across 8 NeuronCores (256 rows each);
    replicate all weights.
  - Host pre-transposes each x shard to [G, 256] (gene-major) and appends a
    ones-row so the encoder bias rides inside the big GEMM; the gene dim is
    zero-padded to a multiple of 128.
  - On device: encoder GEMM (K=10001) accumulates in PSUM with fp32r
    (single-pass FP22-truncated fp32) matmuls, then LN -> ReLU -> second
    encoder GEMM, 3 attention layers (seq len 1 => attention collapses to
    f @ Wv @ Wo + residual + LN), cosine-sim prototype softmax, classifier.
    All activation transposes are done on the PE with an identity matrix.
  - mod_assign softmax row-sums (for sparsity_loss) and marker_weights row
    normalization are sharded across cores by rows.
  - marker_scores is a pure broadcast of marker_weights -> returned as a
    host-side broadcast view (no compute, no copy).
"""
import os
import sys

sys.path.insert(0, '/opt/trn_rl_repo')

import numpy as np

import concourse.bass as bass
import concourse.tile as tile
from concourse import bacc, mybir
from concourse.masks import make_identity

F32 = mybir.dt.float32
F32R = mybir.dt.float32r

N_CORES = 8
B, G, HD, H2, C, S, M, L = 2048, 10000, 256, 512, 16, 32, 100, 3
BC = B // N_CORES              # 256 batch rows per core
GP = 10240                     # (G + 1 bias row) padded up to 20*512
KT = GP // 128                 # 80 k-tiles in the encoder GEMM
KG = 8                         # k-tiles loaded per DMA
NG = KT // KG                  # 20 DMA groups
MROWS = 13                     # mod_assign rows per core (13*8 >= 100)
CROWS = C // N_CORES           # marker rows per core
EPS = 1e-5
STATE_TEMP = 0.5


def _bcast(ap, p=128):
    """Read a DRAM AP replicated across p partitions (stride-0 partition dim)."""
    return bass.AP(tensor=ap.tensor, offset=ap.offset, ap=[[0, p], *ap.ap])


def _build_program():
    nc = bacc.Bacc("TRN2", target_bir_lowering=False, debug=False,
                   num_devices=N_CORES)

    # ---- DRAM I/O ----
    xT = nc.dram_tensor("xT", [GP, BC], F32R, kind="ExternalInput").ap()
    w1a = nc.dram_tensor("w1a", [GP, H2], F32R, kind="ExternalInput").ap()
    eng = nc.dram_tensor("eng", [128, H2], F32, kind="ExternalInput").ap()
    enb = nc.dram_tensor("enb", [128, H2], F32, kind="ExternalInput").ap()
    w2 = nc.dram_tensor("w2", [H2, HD], F32, kind="ExternalInput").ap()
    b2 = nc.dram_tensor("b2", [128, HD], F32, kind="ExternalInput").ap()
    wv = nc.dram_tensor("wv", [L, HD, HD], F32, kind="ExternalInput").ap()
    bv = nc.dram_tensor("bv", [128, L, HD], F32, kind="ExternalInput").ap()
    wo = nc.dram_tensor("wo", [L, HD, HD], F32, kind="ExternalInput").ap()
    bo = nc.dram_tensor("bo", [128, L, HD], F32, kind="ExternalInput").ap()
    lng = nc.dram_tensor("lng", [128, L, HD], F32, kind="ExternalInput").ap()
    lnb = nc.dram_tensor("lnb", [128, L, HD], F32, kind="ExternalInput").ap()
    protos = nc.dram_tensor("protos", [S, HD], F32, kind="ExternalInput").ap()
    cw1 = nc.dram_tensor("cw1", [HD, HD // 2], F32, kind="ExternalInput").ap()
    cb1 = nc.dram_tensor("cb1", [128, HD // 2], F32, kind="ExternalInput").ap()
    cw2 = nc.dram_tensor("cw2", [HD // 2, C], F32, kind="ExternalInput").ap()
    cb2 = nc.dram_tensor("cb2", [128, C], F32, kind="ExternalInput").ap()
    mod = nc.dram_tensor("mod", [MROWS, G], F32, kind="ExternalInput").ap()
    mkw = nc.dram_tensor("mkw", [CROWS, G], F32, kind="ExternalInput").ap()

    o_cs = nc.dram_tensor("o_cs", [BC, HD], F32, kind="ExternalOutput").ap()
    o_lg = nc.dram_tensor("o_lg", [BC, C], F32, kind="ExternalOutput").ap()
    o_ms = nc.dram_tensor("o_ms", [MROWS, 1], F32, kind="ExternalOutput").ap()
    o_mk = nc.dram_tensor("o_mk", [CROWS, G], F32, kind="ExternalOutput").ap()

    with tile.TileContext(nc) as tc:
        with (
            tc.tile_pool(name="const", bufs=1) as const,
            tc.tile_pool(name="xk", bufs=2) as xkp,
            tc.tile_pool(name="wk", bufs=2) as wkp,
            tc.tile_pool(name="work", bufs=3) as work,
            tc.tile_pool(name="act", bufs=3) as actp,
            tc.tile_pool(name="tposed", bufs=2) as tpp,
            tc.tile_pool(name="small", bufs=4) as sm,
            tc.tile_pool(name="modp", bufs=1) as modp,
            tc.tile_pool(name="acc", bufs=1, space="PSUM") as accp,
            tc.tile_pool(name="tp", bufs=2, space="PSUM") as tpsum,
            tc.tile_pool(name="mm", bufs=3, space="PSUM") as mmpsum,
        ):
            # ---------- encoder GEMM: y1[BC, H2] = xT.T @ w1a ----------
            acc = [accp.tile([128, H2], F32, name=f"acc{m}") for m in range(2)]
            xTg = xT.rearrange("(g p t) b -> g p t b", p=128, t=KG)
            w1g = w1a.rearrange("(g p t) n -> g p t n", p=128, t=KG)
            for g in range(NG):
                xk = xkp.tile([128, KG, BC], F32R, name="xk")
                nc.scalar.dma_start(out=xk, in_=xTg[g])
                wk = wkp.tile([128, KG, H2], F32R, name="wk")
                nc.sync.dma_start(out=wk, in_=w1g[g])
                for t in range(KG):
                    for m in range(2):
                        nc.tensor.matmul(acc[m], xk[:, t, m * 128:(m + 1) * 128],
                                         wk[:, t, :],
                                         start=(g == 0 and t == 0),
                                         stop=(g == NG - 1 and t == KG - 1))

            # ---------- constants ----------
            ident = const.tile([128, 128], F32, name="ident")
            make_identity(nc, ident)
            eps_t = const.tile([128, 1], F32, name="eps_t")
            nc.vector.memset(eps_t, EPS)

            w2_sb = const.tile([128, H2 // 128, HD], F32, name="w2_sb")
            nc.sync.dma_start(out=w2_sb, in_=w2.rearrange("(t p) n -> p t n", p=128))
            wv_sb = const.tile([128, L, 2, HD], F32, name="wv_sb")
            nc.sync.dma_start(out=wv_sb, in_=wv.rearrange("l (t p) n -> p l t n", p=128))
            wo_sb = const.tile([128, L, 2, HD], F32, name="wo_sb")
            nc.sync.dma_start(out=wo_sb, in_=wo.rearrange("l (t p) n -> p l t n", p=128))
            cw1_sb = const.tile([128, 2, HD // 2], F32, name="cw1_sb")
            nc.sync.dma_start(out=cw1_sb, in_=cw1.rearrange("(t p) n -> p t n", p=128))
            cw2_sb = const.tile([128, C], F32, name="cw2_sb")
            nc.sync.dma_start(out=cw2_sb, in_=cw2)

            # broadcast bias/scale vectors across partitions
            eng_b = const.tile([128, H2], F32, name="eng_b")
            nc.sync.dma_start(out=eng_b, in_=eng)
            enb_b = const.tile([128, H2], F32, name="enb_b")
            nc.sync.dma_start(out=enb_b, in_=enb)
            b2_b = const.tile([128, HD], F32, name="b2_b")
            nc.sync.dma_start(out=b2_b, in_=b2)
            bv_b = const.tile([128, L, HD], F32, name="bv_b")
            nc.sync.dma_start(out=bv_b, in_=bv)
            bo_b = const.tile([128, L, HD], F32, name="bo_b")
            nc.sync.dma_start(out=bo_b, in_=bo)
            lng_b = const.tile([128, L, HD], F32, name="lng_b")
            nc.sync.dma_start(out=lng_b, in_=lng)
            lnb_b = const.tile([128, L, HD], F32, name="lnb_b")
            nc.sync.dma_start(out=lnb_b, in_=lnb)
            cb1_b = const.tile([128, HD // 2], F32, name="cb1_b")
            nc.sync.dma_start(out=cb1_b, in_=cb1)
            cb2_b = const.tile([128, C], F32, name="cb2_b")
            nc.sync.dma_start(out=cb2_b, in_=cb2)

            # prototypes: raw (zero-padded to 128 partitions) + row-normalized
            # transpose [HD(part-tiles), S]
            pro_sb = const.tile([128, HD], F32, name="pro_sb")
            nc.vector.memset(pro_sb, 0.0)
            nc.sync.dma_start(out=pro_sb[:S, :], in_=protos)
            pn_sq = sm.tile([S, HD], F32, name="pn_sq")
            nc.vector.tensor_mul(pn_sq, pro_sb[:S, :], pro_sb[:S, :])
            pn = sm.tile([S, 1], F32, name="pn")
            nc.vector.reduce_sum(out=pn, in_=pn_sq, axis=mybir.AxisListType.X)
            nc.scalar.activation(out=pn, in_=pn, func=mybir.ActivationFunctionType.Sqrt)
            rpn = sm.tile([S, 1], F32, name="rpn")
            nc.vector.reciprocal(out=rpn, in_=pn)
            pro_n = const.tile([S, HD], F32, name="pro_n")
            nc.vector.tensor_scalar_mul(out=pro_n, in0=pro_sb[:S, :], scalar1=rpn)
            proT = const.tile([128, 2, S], F32, name="proT")
            for j in range(2):
                pt = tpsum.tile([128, 128], F32, name="pt_pro", tag="tps")
                nc.tensor.transpose(pt[:, :S], pro_n[:, j * 128:(j + 1) * 128],
                                    ident[:S, :S])
                nc.vector.tensor_copy(out=proT[:, j, :], in_=pt[:, :S])

            # ---------- mod_assign softmax row-sums ----------
            msh = modp.tile([MROWS, G], F32, name="msh")
            nc.sync.dma_start(out=msh, in_=mod)
            mmax = sm.tile([MROWS, 1], F32, name="mmax")
            nc.vector.reduce_max(out=mmax, in_=msh, axis=mybir.AxisListType.X)
            nmmax = sm.tile([MROWS, 1], F32, name="nmmax")
            nc.vector.tensor_scalar_mul(out=nmmax, in0=mmax, scalar1=-1.0)
            nc.scalar.activation(out=msh, in_=msh,
                                 func=mybir.ActivationFunctionType.Exp, bias=nmmax)
            msum = sm.tile([MROWS, 1], F32, name="msum")
            nc.vector.reduce_sum(out=msum, in_=msh, axis=mybir.AxisListType.X)
            rmsum = sm.tile([MROWS, 1], F32, name="rmsum")
            nc.vector.reciprocal(out=rmsum, in_=msum)
            nc.vector.tensor_scalar_mul(out=msh, in0=msh, scalar1=rmsum)
            msum2 = sm.tile([MROWS, 1], F32, name="msum2")
            nc.vector.reduce_sum(out=msum2, in_=msh, axis=mybir.AxisListType.X)
            nc.sync.dma_start(out=o_ms, in_=msum2)

            # ---------- marker importance ----------
            mk = modp.tile([CROWS, G], F32, name="mk")
            nc.sync.dma_start(out=mk, in_=mkw)
            mks = sm.tile([CROWS, 1], F32, name="mks")
            nc.vector.reduce_sum(out=mks, in_=mk, axis=mybir.AxisListType.X,
                                 apply_absolute_value=True)
            nc.vector.tensor_scalar_max(out=mks, in0=mks, scalar1=1e-12)
            rmks = sm.tile([CROWS, 1], F32, name="rmks")
            nc.vector.reciprocal(out=rmks, in_=mks)
            nc.vector.tensor_scalar_mul(out=mk, in0=mk, scalar1=rmks)
            nc.sync.dma_start(out=o_mk, in_=mk)

            def layer_norm(x_sb_or_psum, out_sb, g_ap, b_ap, width):
                stats = sm.tile([128, 6], F32, name="lnstats")
                nc.vector.bn_stats(out=stats, in_=x_sb_or_psum)
                mv = sm.tile([128, 2], F32, name="lnmv")
                nc.vector.bn_aggr(out=mv, in_=stats)
                std = sm.tile([128, 1], F32, name="lnstd")
                nc.scalar.activation(out=std, in_=mv[:, 1:2],
                                     func=mybir.ActivationFunctionType.Sqrt,
                                     bias=eps_t)
                rstd = sm.tile([128, 1], F32, name="lnrstd")
                nc.vector.reciprocal(out=rstd, in_=std)
                nc.vector.tensor_scalar(out=out_sb, in0=x_sb_or_psum,
                                        scalar1=mv[:, 0:1], scalar2=rstd,
                                        op0=mybir.AluOpType.subtract,
                                        op1=mybir.AluOpType.mult)
                nc.vector.tensor_mul(out_sb, out_sb, g_ap)
                nc.vector.tensor_add(out_sb, out_sb, b_ap)

            def transpose_to(src_sb, n_blocks, name):
                """src [128, n_blocks*128] -> SBUF [128, n_blocks, 128]."""
                dst = tpp.tile([128, n_blocks, 128], F32, name=name)
                for j in range(n_blocks):
                    pt = tpsum.tile([128, 128], F32, name="pt_" + name, tag="tps")
                    nc.tensor.transpose(pt, src_sb[:, j * 128:(j + 1) * 128], ident)
                    nc.vector.tensor_copy(out=dst[:, j, :], in_=pt)
                return dst

            for m in range(2):
                rows = slice(m * 128, (m + 1) * 128)
                # LN -> *g -> +b -> ReLU
                h = work.tile([128, H2], F32, name="h")
                layer_norm(acc[m], h, eng_b, enb_b, H2)
                nc.scalar.activation(out=h, in_=h,
                                     func=mybir.ActivationFunctionType.Relu)

                # gene_features = h @ w2 + b2
                hT = transpose_to(h, 4, "hT")
                gf_ps = mmpsum.tile([128, HD], F32, name="gf_ps", tag="mmps")
                for j in range(4):
                    nc.tensor.matmul(gf_ps, hT[:, j, :], w2_sb[:, j, :],
                                     start=(j == 0), stop=(j == 3))
                f = actp.tile([128, HD], F32, name="f")
                nc.vector.tensor_add(f, gf_ps, b2_b)

                # attention layers (seq len 1): f = LN(f @ Wv @ Wo + biases + f)
                for l in range(L):
                    fT = transpose_to(f, 2, "fT")
                    o1_ps = mmpsum.tile([128, HD], F32, name="o1_ps", tag="mmps")
                    for j in range(2):
                        nc.tensor.matmul(o1_ps, fT[:, j, :],
                                         wv_sb[:, l, j, :],
                                         start=(j == 0), stop=(j == 1))
                    o1 = actp.tile([128, HD], F32, name="o1")
                    nc.vector.tensor_add(o1, o1_ps, bv_b[:, l, :])
                    o1T = transpose_to(o1, 2, "o1T")
                    o2_ps = mmpsum.tile([128, HD], F32, name="o2_ps", tag="mmps")
                    for j in range(2):
                        nc.tensor.matmul(o2_ps, o1T[:, j, :],
                                         wo_sb[:, l, j, :],
                                         start=(j == 0), stop=(j == 1))
                    o2 = actp.tile([128, HD], F32, name="o2")
                    nc.vector.tensor_add(o2, o2_ps, bo_b[:, l, :])
                    nc.vector.tensor_add(o2, o2, f)
                    f = actp.tile([128, HD], F32, name="f")
                    layer_norm(o2, f, lng_b[:, l, :], lnb_b[:, l, :], HD)

                # cosine-sim prototype softmax
                fsq = work.tile([128, HD], F32, name="fsq")
                nc.vector.tensor_mul(fsq, f, f)
                fn = sm.tile([128, 1], F32, name="fn")
                nc.vector.reduce_sum(out=fn, in_=fsq, axis=mybir.AxisListType.X)
                nc.scalar.activation(out=fn, in_=fn,
                                     func=mybir.ActivationFunctionType.Sqrt)
                rfn = sm.tile([128, 1], F32, name="rfn")
                nc.vector.reciprocal(out=rfn, in_=fn)
                rfn2 = sm.tile([128, 1], F32, name="rfn2")
                nc.vector.tensor_scalar_mul(out=rfn2, in0=rfn,
                                            scalar1=1.0 / STATE_TEMP)
                fT = transpose_to(f, 2, "fT2")
                sim_ps = mmpsum.tile([128, HD], F32, name="sim_ps", tag="mmps")[:, :S]
                for j in range(2):
                    nc.tensor.matmul(sim_ps, fT[:, j, :], proT[:, j, :],
                                     start=(j == 0), stop=(j == 1))
                sw = work.tile([128, S], F32, name="sw")
                nc.vector.tensor_scalar_mul(out=sw, in0=sim_ps, scalar1=rfn2)
                smx = sm.tile([128, 1], F32, name="smx")
                nc.vector.reduce_max(out=smx, in_=sw, axis=mybir.AxisListType.X)
                nsmx = sm.tile([128, 1], F32, name="nsmx")
                nc.vector.tensor_scalar_mul(out=nsmx, in0=smx, scalar1=-1.0)
                nc.scalar.activation(out=sw, in_=sw,
                                     func=mybir.ActivationFunctionType.Exp,
                                     bias=nsmx)
                ssum = sm.tile([128, 1], F32, name="ssum")
                nc.vector.reduce_sum(out=ssum, in_=sw, axis=mybir.AxisListType.X)
                rssum = sm.tile([128, 1], F32, name="rssum")
                nc.vector.reciprocal(out=rssum, in_=ssum)
                nc.vector.tensor_scalar_mul(out=sw, in0=sw, scalar1=rssum)

                # cell_states = sw @ prototypes  (pad K=32 -> 128 with zeros)
                swT = tpp.tile([128, 128], F32, name="swT")
                nc.vector.memset(swT, 0.0)
                swT_ps = tpsum.tile([128, 128], F32, name="swT_ps", tag="tps")
                nc.tensor.transpose(swT_ps[:S, :], sw, ident)
                nc.vector.tensor_copy(out=swT[:S, :], in_=swT_ps[:S, :])
                cs_ps = mmpsum.tile([128, HD], F32, name="cs_ps", tag="mmps")
                nc.tensor.matmul(cs_ps, swT, pro_sb, start=True, stop=True)
                cs = actp.tile([128, HD], F32, name="cs")
                nc.vector.tensor_copy(out=cs, in_=cs_ps)
                nc.sync.dma_start(out=o_cs[rows, :], in_=cs)

                # classifier head
                csT = transpose_to(cs, 2, "csT")
                h1_ps = mmpsum.tile([128, HD], F32, name="h1_ps", tag="mmps")[:, :HD // 2]
                for j in range(2):
                    nc.tensor.matmul(h1_ps, csT[:, j, :], cw1_sb[:, j, :],
                                     start=(j == 0), stop=(j == 1))
                h1 = work.tile([128, HD // 2], F32, name="h1")
                nc.vector.tensor_add(h1, h1_ps, cb1_b)
                nc.scalar.activation(out=h1, in_=h1,
                                     func=mybir.ActivationFunctionType.Relu)
                h1T = transpose_to(h1, 1, "h1T")
                lg_ps = mmpsum.tile([128, HD], F32, name="lg_ps", tag="mmps")[:, :C]
                nc.tensor.matmul(lg_ps, h1T[:, 0, :], cw2_sb,
                                 start=True, stop=True)
                lg = work.tile([128, C], F32, name="lg")
                nc.vector.tensor_add(lg, lg_ps, cb2_b)
                nc.sync.dma_start(out=o_lg[rows, :], in_=lg)

    nc.finalize()
    return nc


_CACHE = {}


def get_program():
    if "nc" not in _CACHE:
        _CACHE["nc"] = _build_program()
    return _CACHE["nc"]


def make_in_maps(inputs):
    """Build the 8 per-core input dicts from the full-size input dict."""
    x = np.asarray(inputs["x"], dtype=np.float32)
    w1 = np.asarray(inputs["enc_w1"], dtype=np.float32)
    w1a = np.zeros((GP, H2), dtype=np.float32)
    w1a[:G] = w1
    w1a[G] = np.asarray(inputs["enc_b1"], dtype=np.float32)

    qkv_w = np.asarray(inputs["qkv_w"], dtype=np.float32)
    qkv_b = np.asarray(inputs["qkv_b"], dtype=np.float32)
    def rep(v):
        v = np.asarray(v, dtype=np.float32)
        return np.ascontiguousarray(np.broadcast_to(v[None], (128,) + v.shape))

    common = {
        "w1a": w1a,
        "eng": rep(inputs["enc_ln_g"]),
        "enb": rep(inputs["enc_ln_b"]),
        "w2": np.asarray(inputs["enc_w2"], dtype=np.float32),
        "b2": rep(inputs["enc_b2"]),
        "wv": np.ascontiguousarray(qkv_w[:, :, 2 * HD:3 * HD]),
        "bv": rep(qkv_b[:, 2 * HD:3 * HD]),
        "wo": np.asarray(inputs["out_w"], dtype=np.float32),
        "bo": rep(inputs["out_b"]),
        "lng": rep(inputs["ln_g"]),
        "lnb": rep(inputs["ln_b"]),
        "protos": np.asarray(inputs["prototypes"], dtype=np.float32),
        "cw1": np.asarray(inputs["clf_w1"], dtype=np.float32),
        "cb1": rep(inputs["clf_b1"]),
        "cw2": np.asarray(inputs["clf_w2"], dtype=np.float32),
        "cb2": rep(inputs["clf_b2"]),
    }
    mod_assign = np.asarray(inputs["mod_assign"], dtype=np.float32)
    marker = np.asarray(inputs["marker_weights"], dtype=np.float32)

    in_maps = []
    for c in range(N_CORES):
        xs = x[c * BC:(c + 1) * BC, :]          # [BC, G]
        xTc = np.zeros((GP, BC), dtype=np.float32)
        xTc[:G] = xs.T
        xTc[G] = 1.0
        mrows = np.minimum(np.arange(c * MROWS, (c + 1) * MROWS), M - 1)
        in_maps.append({
            "xT": np.ascontiguousarray(xTc),
            "mod": np.ascontiguousarray(mod_assign[mrows]),
            "mkw": np.ascontiguousarray(marker[c * CROWS:(c + 1) * CROWS]),
            **common,
        })
    return in_maps


LAST_RESULTS = None


def kernel(**inputs):
    global LAST_RESULTS
    from concourse.bass_utils import run_bass_kernel_spmd

    nc = get_program()
    in_maps = make_in_maps(inputs)

    trace = bool(int(os.environ.get("KERNEL_TRACE", "0")))
    if trace:
        import profhook
        profhook.install()

    res = run_bass_kernel_spmd(nc, in_maps, core_ids=list(range(N_CORES)),
                               trace=trace)
    LAST_RESULTS = res

    logits = np.concatenate([res.results[c]["o_lg"] for c in range(N_CORES)], axis=0)
    cell_states = np.concatenate([res.results[c]["o_cs"] for c in range(N_CORES)],
                                 axis=0)
    rowsums = np.concatenate([res.results[c]["o_ms"][:, 0] for c in range(N_CORES)])
    sparsity_loss = np.float32(rowsums[:M].sum() / (M * G) * 0.01)
    marker_importance = np.concatenate([res.results[c]["o_mk"]
                                        for c in range(N_CORES)], axis=0)
    marker = np.asarray(inputs["marker_weights"], dtype=np.float32)
    marker_scores = np.broadcast_to(marker[None], (B, C, G))
    return (logits, cell_states, sparsity_loss, marker_scores, marker_importance)


# revision 10
# speedup vs baseline: 1.0502x; 1.0502x over previous
"""Trainium2 Bass kernel for the BioHAN model (8-core data-parallel SPMD).

Strategy:
  - Shard the batch dim (B=2048) across 8 NeuronCores (256 rows each);
    replicate all weights.
  - Host pre-transposes each x shard to [G, 256] (gene-major) and appends a
    ones-row so the encoder bias rides inside the big GEMM; the gene dim is
    zero-padded to a multiple of 128.
  - On device: encoder GEMM (K=10001) accumulates in PSUM with fp32r
    (single-pass FP22-truncated fp32) matmuls, then LN -> ReLU -> second
    encoder GEMM, 3 attention layers (seq len 1 => attention collapses to
    f @ Wv @ Wo + residual + LN), cosine-sim prototype softmax, classifier.
    All activation transposes are done on the PE with an identity matrix.
  - mod_assign softmax row-sums (for sparsity_loss) and marker_weights row
    normalization are sharded across cores by rows.
  - marker_scores is a pure broadcast of marker_weights -> returned as a
    host-side broadcast view (no compute, no copy).
"""
import os
import sys

sys.path.insert(0, '/opt/trn_rl_repo')

import numpy as np

import concourse.bass as bass
import concourse.tile as tile
from concourse import bacc, mybir
from concourse.masks import make_identity

F32 = mybir.dt.float32
F32R = mybir.dt.float32r

N_CORES = 8
B, G, HD, H2, C, S, M, L = 2048, 10000, 256, 512, 16, 32, 100, 3
BC = B // N_CORES              # 256 batch rows per core
GP = 10240                     # (G + 1 bias row) padded up to 20*512
KT = GP // 128                 # 80 k-tiles in the encoder GEMM
KG = 8                         # k-tiles loaded per DMA
NG = KT // KG                  # 20 DMA groups
MROWS = 13                     # mod_assign rows per core (13*8 >= 100)
CROWS = C // N_CORES           # marker rows per core
EPS = 1e-5
STATE_TEMP = 0.5


def _bcast(ap, p=128):
    """Read a DRAM AP replicated across p partitions (stride-0 partition dim)."""
    return bass.AP(tensor=ap.tensor, offset=ap.offset, ap=[[0, p], *ap.ap])


def _build_program():
    nc = bacc.Bacc("TRN2", target_bir_lowering=False, debug=False,
                   num_devices=N_CORES)

    # ---- DRAM I/O ----
    xT = nc.dram_tensor("xT", [GP, BC], F32R, kind="ExternalInput").ap()
    w1a = nc.dram_tensor("w1a", [GP, H2], F32R, kind="ExternalInput").ap()
    eng = nc.dram_tensor("eng", [128, H2], F32, kind="ExternalInput").ap()
    enb = nc.dram_tensor("enb", [128, H2], F32, kind="ExternalInput").ap()
    w2 = nc.dram_tensor("w2", [H2, HD], F32, kind="ExternalInput").ap()
    b2 = nc.dram_tensor("b2", [128, HD], F32, kind="ExternalInput").ap()
    wv = nc.dram_tensor("wv", [L, HD, HD], F32, kind="ExternalInput").ap()
    bv = nc.dram_tensor("bv", [128, L, HD], F32, kind="ExternalInput").ap()
    wo = nc.dram_tensor("wo", [L, HD, HD], F32, kind="ExternalInput").ap()
    bo = nc.dram_tensor("bo", [128, L, HD], F32, kind="ExternalInput").ap()
    lng = nc.dram_tensor("lng", [128, L, HD], F32, kind="ExternalInput").ap()
    lnb = nc.dram_tensor("lnb", [128, L, HD], F32, kind="ExternalInput").ap()
    protos = nc.dram_tensor("protos", [S, HD], F32, kind="ExternalInput").ap()
    cw1 = nc.dram_tensor("cw1", [HD, HD // 2], F32, kind="ExternalInput").ap()
    cb1 = nc.dram_tensor("cb1", [128, HD // 2], F32, kind="ExternalInput").ap()
    cw2 = nc.dram_tensor("cw2", [HD // 2, C], F32, kind="ExternalInput").ap()
    cb2 = nc.dram_tensor("cb2", [128, C], F32, kind="ExternalInput").ap()
    mod = nc.dram_tensor("mod", [MROWS * 8, G // 8], F32, kind="ExternalInput").ap()
    bdiag = nc.dram_tensor("bdiag", [128, 128], F32, kind="ExternalInput").ap()
    mkw = nc.dram_tensor("mkw", [CROWS * 8, G // 8], F32, kind="ExternalInput").ap()

    o_cs = nc.dram_tensor("o_cs", [BC, HD], F32, kind="ExternalOutput").ap()
    o_lg = nc.dram_tensor("o_lg", [BC, C], F32, kind="ExternalOutput").ap()
    o_ms = nc.dram_tensor("o_ms", [MROWS * 8, 1], F32, kind="ExternalOutput").ap()
    o_mk = nc.dram_tensor("o_mk", [CROWS * 8, G // 8], F32, kind="ExternalOutput").ap()

    with tile.TileContext(nc) as tc:
        with (
            tc.tile_pool(name="const", bufs=1) as const,
            tc.tile_pool(name="xk", bufs=2) as xkp,
            tc.tile_pool(name="wk", bufs=2) as wkp,
            tc.tile_pool(name="work", bufs=3) as work,
            tc.tile_pool(name="act", bufs=3) as actp,
            tc.tile_pool(name="tposed", bufs=2) as tpp,
            tc.tile_pool(name="small", bufs=4) as sm,
            tc.tile_pool(name="modp", bufs=1) as modp,
            tc.tile_pool(name="acc", bufs=1, space="PSUM") as accp,
            tc.tile_pool(name="tp", bufs=2, space="PSUM") as tpsum,
            tc.tile_pool(name="mm", bufs=3, space="PSUM") as mmpsum,
        ):
            # ---------- constants ----------
            ident = const.tile([128, 128], F32, name="ident")
            make_identity(nc, ident)
            eps_t = const.tile([128, 1], F32, name="eps_t")
            nc.vector.memset(eps_t, EPS)

            w2_sb = const.tile([128, H2 // 128, HD], F32, name="w2_sb")
            nc.gpsimd.dma_start(out=w2_sb, in_=w2.rearrange("(t p) n -> p t n", p=128))
            wv_sb = const.tile([128, L, 2, HD], F32, name="wv_sb")
            nc.gpsimd.dma_start(out=wv_sb, in_=wv.rearrange("l (t p) n -> p l t n", p=128))
            wo_sb = const.tile([128, L, 2, HD], F32, name="wo_sb")
            nc.gpsimd.dma_start(out=wo_sb, in_=wo.rearrange("l (t p) n -> p l t n", p=128))
            cw1_sb = const.tile([128, 2, HD // 2], F32, name="cw1_sb")
            nc.gpsimd.dma_start(out=cw1_sb, in_=cw1.rearrange("(t p) n -> p t n", p=128))
            cw2_sb = const.tile([128, C], F32, name="cw2_sb")
            nc.gpsimd.dma_start(out=cw2_sb, in_=cw2)

            # broadcast bias/scale vectors across partitions
            eng_b = const.tile([128, H2], F32, name="eng_b")
            nc.gpsimd.dma_start(out=eng_b, in_=eng)
            enb_b = const.tile([128, H2], F32, name="enb_b")
            nc.gpsimd.dma_start(out=enb_b, in_=enb)
            b2_b = const.tile([128, HD], F32, name="b2_b")
            nc.gpsimd.dma_start(out=b2_b, in_=b2)
            bv_b = const.tile([128, L, HD], F32, name="bv_b")
            nc.gpsimd.dma_start(out=bv_b, in_=bv)
            bo_b = const.tile([128, L, HD], F32, name="bo_b")
            nc.gpsimd.dma_start(out=bo_b, in_=bo)
            lng_b = const.tile([128, L, HD], F32, name="lng_b")
            nc.gpsimd.dma_start(out=lng_b, in_=lng)
            lnb_b = const.tile([128, L, HD], F32, name="lnb_b")
            nc.gpsimd.dma_start(out=lnb_b, in_=lnb)
            cb1_b = const.tile([128, HD // 2], F32, name="cb1_b")
            nc.gpsimd.dma_start(out=cb1_b, in_=cb1)
            cb2_b = const.tile([128, C], F32, name="cb2_b")
            nc.gpsimd.dma_start(out=cb2_b, in_=cb2)

            # prototypes: raw (zero-padded to 128 partitions) + row-normalized
            # transpose [HD(part-tiles), S]
            pro_sb = const.tile([128, HD], F32, name="pro_sb")
            nc.vector.memset(pro_sb, 0.0)
            nc.gpsimd.dma_start(out=pro_sb[:S, :], in_=protos)
            pn_sq = sm.tile([S, HD], F32, name="pn_sq")
            nc.vector.tensor_mul(pn_sq, pro_sb[:S, :], pro_sb[:S, :])
            pn = sm.tile([S, 1], F32, name="pn")
            nc.vector.reduce_sum(out=pn, in_=pn_sq, axis=mybir.AxisListType.X)
            nc.scalar.activation(out=pn, in_=pn, func=mybir.ActivationFunctionType.Sqrt)
            rpn = sm.tile([S, 1], F32, name="rpn")
            nc.vector.reciprocal(out=rpn, in_=pn)
            pro_n = const.tile([S, HD], F32, name="pro_n")
            nc.vector.tensor_scalar_mul(out=pro_n, in0=pro_sb[:S, :], scalar1=rpn)
            proT = const.tile([128, 2, S], F32, name="proT")
            for j in range(2):
                pt = tpsum.tile([128, 128], F32, name="pt_pro", tag="tps")
                nc.tensor.transpose(pt[:, :S], pro_n[:, j * 128:(j + 1) * 128],
                                    ident[:S, :S])
                nc.vector.tensor_copy(out=proT[:, j, :], in_=pt[:, :S])

            # ---------- mod_assign softmax row-sums (rows spread 8-wide) ----------
            MP = MROWS * 8
            bd_sb = const.tile([128, 128], F32, name="bd_sb")
            nc.gpsimd.dma_start(out=bd_sb, in_=bdiag)
            msh = modp.tile([MP, G // 8], F32, name="msh")
            nc.gpsimd.dma_start(out=msh, in_=mod)
            nc.scalar.activation(out=msh, in_=msh,
                                 func=mybir.ActivationFunctionType.Exp)
            mcs = sm.tile([MP, 1], F32, name="mcs")
            nc.vector.reduce_sum(out=mcs, in_=msh, axis=mybir.AxisListType.X)
            mden_ps = tpsum.tile([128, 128], F32, name="mden_ps", tag="tps")
            nc.tensor.matmul(mden_ps[:MP, :1], bd_sb[:MP, :MP], mcs,
                             start=True, stop=True)
            rmden = sm.tile([MP, 1], F32, name="rmden")
            nc.vector.reciprocal(out=rmden, in_=mden_ps[:MP, :1])
            nc.vector.tensor_scalar_mul(out=msh, in0=msh, scalar1=rmden)
            msum2 = sm.tile([MP, 1], F32, name="msum2")
            nc.vector.reduce_sum(out=msum2, in_=msh, axis=mybir.AxisListType.X)
            nc.sync.dma_start(out=o_ms, in_=msum2)

            # ---------- marker importance (rows spread 8-wide) ----------
            CP = CROWS * 8
            mk = modp.tile([CP, G // 8], F32, name="mk")
            nc.gpsimd.dma_start(out=mk, in_=mkw)
            mkcs = sm.tile([CP, 1], F32, name="mkcs")
            nc.vector.reduce_sum(out=mkcs, in_=mk, axis=mybir.AxisListType.X,
                                 apply_absolute_value=True)
            mkden_ps = tpsum.tile([128, 128], F32, name="mkden_ps", tag="tps")
            nc.tensor.matmul(mkden_ps[:CP, :1], bd_sb[:CP, :CP], mkcs,
                             start=True, stop=True)
            mkden = sm.tile([CP, 1], F32, name="mkden")
            nc.vector.tensor_scalar_max(out=mkden, in0=mkden_ps[:CP, :1],
                                        scalar1=1e-12)
            rmks = sm.tile([CP, 1], F32, name="rmks")
            nc.vector.reciprocal(out=rmks, in_=mkden)
            nc.vector.tensor_scalar_mul(out=mk, in0=mk, scalar1=rmks)
            nc.sync.dma_start(out=o_mk, in_=mk)

            # ---------- encoder GEMM: y1[BC, H2] = xT.T @ w1a ----------
            acc = [accp.tile([128, H2], F32, name=f"acc{m}") for m in range(2)]
            xTg = xT.rearrange("(g p t) b -> g p t b", p=128, t=KG)
            w1g = w1a.rearrange("(g p t) n -> g p t n", p=128, t=KG)
            for g in range(NG):
                xk = xkp.tile([128, KG, BC], F32R, name="xk")
                nc.scalar.dma_start(out=xk, in_=xTg[g])
                wk = wkp.tile([128, KG, H2], F32R, name="wk")
                nc.sync.dma_start(out=wk, in_=w1g[g])
                for t in range(KG):
                    for m in range(2):
                        nc.tensor.matmul(acc[m], xk[:, t, m * 128:(m + 1) * 128],
                                         wk[:, t, :],
                                         start=(g == 0 and t == 0),
                                         stop=(g == NG - 1 and t == KG - 1))

            def layer_norm(x_sb_or_psum, out_sb, g_ap, b_ap, width):
                stats = sm.tile([128, 6], F32, name="lnstats")
                nc.vector.bn_stats(out=stats, in_=x_sb_or_psum)
                mv = sm.tile([128, 2], F32, name="lnmv")
                nc.vector.bn_aggr(out=mv, in_=stats)
                std = sm.tile([128, 1], F32, name="lnstd")
                nc.scalar.activation(out=std, in_=mv[:, 1:2],
                                     func=mybir.ActivationFunctionType.Sqrt,
                                     bias=eps_t)
                rstd = sm.tile([128, 1], F32, name="lnrstd")
                nc.vector.reciprocal(out=rstd, in_=std)
                nc.vector.tensor_scalar(out=out_sb, in0=x_sb_or_psum,
                                        scalar1=mv[:, 0:1], scalar2=rstd,
                                        op0=mybir.AluOpType.subtract,
                                        op1=mybir.AluOpType.mult)
                nc.vector.tensor_mul(out_sb, out_sb, g_ap)
                nc.vector.tensor_add(out_sb, out_sb, b_ap)

            def transpose_to(src_sb, n_blocks, name):
                """src [128, n_blocks*128] -> SBUF [128, n_blocks, 128]."""
                dst = tpp.tile([128, n_blocks, 128], F32, name=name)
                for j in range(n_blocks):
                    pt = tpsum.tile([128, 128], F32, name="pt_" + name, tag="tps")
                    nc.tensor.transpose(pt, src_sb[:, j * 128:(j + 1) * 128], ident)
                    nc.vector.tensor_copy(out=dst[:, j, :], in_=pt)
                return dst

            for m in range(2):
                rows = slice(m * 128, (m + 1) * 128)
                # LN -> *g -> +b -> ReLU
                h = work.tile([128, H2], F32, name="h")
                layer_norm(acc[m], h, eng_b, enb_b, H2)
                nc.scalar.activation(out=h, in_=h,
                                     func=mybir.ActivationFunctionType.Relu)

                # gene_features = h @ w2 + b2
                hT = transpose_to(h, 4, "hT")
                gf_ps = mmpsum.tile([128, HD], F32, name="gf_ps", tag="mmps")
                for j in range(4):
                    nc.tensor.matmul(gf_ps, hT[:, j, :], w2_sb[:, j, :],
                                     start=(j == 0), stop=(j == 3))
                f = actp.tile([128, HD], F32, name="f")
                nc.vector.tensor_add(f, gf_ps, b2_b)

                # attention layers (seq len 1): f = LN(f @ Wv @ Wo + biases + f)
                for l in range(L):
                    fT = transpose_to(f, 2, "fT")
                    o1_ps = mmpsum.tile([128, HD], F32, name="o1_ps", tag="mmps")
                    for j in range(2):
                        nc.tensor.matmul(o1_ps, fT[:, j, :],
                                         wv_sb[:, l, j, :],
                                         start=(j == 0), stop=(j == 1))
                    o1 = actp.tile([128, HD], F32, name="o1")
                    nc.vector.tensor_add(o1, o1_ps, bv_b[:, l, :])
                    o1T = transpose_to(o1, 2, "o1T")
                    o2_ps = mmpsum.tile([128, HD], F32, name="o2_ps", tag="mmps")
                    for j in range(2):
                        nc.tensor.matmul(o2_ps, o1T[:, j, :],
                                         wo_sb[:, l, j, :],
                                         start=(j == 0), stop=(j == 1))
                    o2 = actp.tile([128, HD], F32, name="o2")
                    nc.vector.tensor_add(o2, o2_ps, bo_b[:, l, :])
                    nc.vector.tensor_add(o2, o2, f)
                    f = actp.tile([128, HD], F32, name="f")
                    layer_norm(o2, f, lng_b[:, l, :], lnb_b[:, l, :], HD)

                # cosine-sim prototype softmax
                fsq = work.tile([128, HD], F32, name="fsq")
                nc.vector.tensor_mul(fsq, f, f)
                fn = sm.tile([128, 1], F32, name="fn")
                nc.vector.reduce_sum(out=fn, in_=fsq, axis=mybir.AxisListType.X)
                nc.scalar.activation(out=fn, in_=fn,
                                     func=mybir.ActivationFunctionType.Sqrt)
                rfn = sm.tile([128, 1], F32, name="rfn")
                nc.vector.reciprocal(out=rfn, in_=fn)
                rfn2 = sm.tile([128, 1], F32, name="rfn2")
                nc.vector.tensor_scalar_mul(out=rfn2, in0=rfn,
                                            scalar1=1.0 / STATE_TEMP)
                fT = transpose_to(f, 2, "fT2")
                sim_ps = mmpsum.tile([128, HD], F32, name="sim_ps", tag="mmps")[:, :S]
                for j in range(2):
                    nc.tensor.matmul(sim_ps, fT[:, j, :], proT[:, j, :],
                                     start=(j == 0), stop=(j == 1))
                sw = work.tile([128, S], F32, name="sw")
                nc.vector.tensor_scalar_mul(out=sw, in0=sim_ps, scalar1=rfn2)
                smx = sm.tile([128, 1], F32, name="smx")
                nc.vector.reduce_max(out=smx, in_=sw, axis=mybir.AxisListType.X)
                nsmx = sm.tile([128, 1], F32, name="nsmx")
                nc.vector.tensor_scalar_mul(out=nsmx, in0=smx, scalar1=-1.0)
                nc.scalar.activation(out=sw, in_=sw,
                                     func=mybir.ActivationFunctionType.Exp,
                                     bias=nsmx)
                ssum = sm.tile([128, 1], F32, name="ssum")
                nc.vector.reduce_sum(out=ssum, in_=sw, axis=mybir.AxisListType.X)
                rssum = sm.tile([128, 1], F32, name="rssum")
                nc.vector.reciprocal(out=rssum, in_=ssum)
                nc.vector.tensor_scalar_mul(out=sw, in0=sw, scalar1=rssum)

                # cell_states = sw @ prototypes  (pad K=32 -> 128 with zeros)
                swT = tpp.tile([128, 128], F32, name="swT")
                nc.vector.memset(swT, 0.0)
                swT_ps = tpsum.tile([128, 128], F32, name="swT_ps", tag="tps")
                nc.tensor.transpose(swT_ps[:S, :], sw, ident)
                nc.vector.tensor_copy(out=swT[:S, :], in_=swT_ps[:S, :])
                cs_ps = mmpsum.tile([128, HD], F32, name="cs_ps", tag="mmps")
                nc.tensor.matmul(cs_ps, swT, pro_sb, start=True, stop=True)
                cs = actp.tile([128, HD], F32, name="cs")
                nc.vector.tensor_copy(out=cs, in_=cs_ps)
                nc.sync.dma_start(out=o_cs[rows, :], in_=cs)

                # classifier head
                csT = transpose_to(cs, 2, "csT")
                h1_ps = mmpsum.tile([128, HD], F32, name="h1_ps", tag="mmps")[:, :HD // 2]
                for j in range(2):
                    nc.tensor.matmul(h1_ps, csT[:, j, :], cw1_sb[:, j, :],
                                     start=(j == 0), stop=(j == 1))
                h1 = work.tile([128, HD // 2], F32, name="h1")
                nc.vector.tensor_add(h1, h1_ps, cb1_b)
                nc.scalar.activation(out=h1, in_=h1,
                                     func=mybir.ActivationFunctionType.Relu)
                h1T = transpose_to(h1, 1, "h1T")
                lg_ps = mmpsum.tile([128, HD], F32, name="lg_ps", tag="mmps")[:, :C]
                nc.tensor.matmul(lg_ps, h1T[:, 0, :], cw2_sb,
                                 start=True, stop=True)
                lg = work.tile([128, C], F32, name="lg")
                nc.vector.tensor_add(lg, lg_ps, cb2_b)
                nc.sync.dma_start(out=o_lg[rows, :], in_=lg)

    nc.finalize()
    return nc


_CACHE = {}


def get_program():
    if "nc" not in _CACHE:
        _CACHE["nc"] = _build_program()
    return _CACHE["nc"]


def make_in_maps(inputs):
    """Build the 8 per-core input dicts from the full-size input dict."""
    x = np.asarray(inputs["x"], dtype=np.float32)
    w1 = np.asarray(inputs["enc_w1"], dtype=np.float32)
    w1a = np.zeros((GP, H2), dtype=np.float32)
    w1a[:G] = w1
    w1a[G] = np.asarray(inputs["enc_b1"], dtype=np.float32)

    qkv_w = np.asarray(inputs["qkv_w"], dtype=np.float32)
    qkv_b = np.asarray(inputs["qkv_b"], dtype=np.float32)
    def rep(v):
        v = np.asarray(v, dtype=np.float32)
        return np.ascontiguousarray(np.broadcast_to(v[None], (128,) + v.shape))

    common = {
        "w1a": w1a,
        "eng": rep(inputs["enc_ln_g"]),
        "enb": rep(inputs["enc_ln_b"]),
        "w2": np.asarray(inputs["enc_w2"], dtype=np.float32),
        "b2": rep(inputs["enc_b2"]),
        "wv": np.ascontiguousarray(qkv_w[:, :, 2 * HD:3 * HD]),
        "bv": rep(qkv_b[:, 2 * HD:3 * HD]),
        "wo": np.asarray(inputs["out_w"], dtype=np.float32),
        "bo": rep(inputs["out_b"]),
        "lng": rep(inputs["ln_g"]),
        "lnb": rep(inputs["ln_b"]),
        "protos": np.asarray(inputs["prototypes"], dtype=np.float32),
        "cw1": np.asarray(inputs["clf_w1"], dtype=np.float32),
        "cb1": rep(inputs["clf_b1"]),
        "cw2": np.asarray(inputs["clf_w2"], dtype=np.float32),
        "cb2": rep(inputs["clf_b2"]),
    }
    mod_assign = np.asarray(inputs["mod_assign"], dtype=np.float32)
    marker = np.asarray(inputs["marker_weights"], dtype=np.float32)
    bdiag = np.kron(np.eye(16, dtype=np.float32), np.ones((8, 8), np.float32))

    in_maps = []
    for c in range(N_CORES):
        xs = x[c * BC:(c + 1) * BC, :]          # [BC, G]
        xTc = np.zeros((GP, BC), dtype=np.float32)
        xTc[:G] = xs.T
        xTc[G] = 1.0
        mrows = np.minimum(np.arange(c * MROWS, (c + 1) * MROWS), M - 1)
        in_maps.append({
            "xT": np.ascontiguousarray(xTc),
            "mod": np.ascontiguousarray(
                mod_assign[mrows].reshape(MROWS * 8, G // 8)),
            "mkw": np.ascontiguousarray(
                marker[c * CROWS:(c + 1) * CROWS].reshape(CROWS * 8, G // 8)),
            "bdiag": bdiag,
            **common,
        })
    return in_maps


LAST_RESULTS = None


def kernel(**inputs):
    global LAST_RESULTS
    from concourse.bass_utils import run_bass_kernel_spmd

    nc = get_program()
    in_maps = make_in_maps(inputs)

    trace = bool(int(os.environ.get("KERNEL_TRACE", "0")))
    if trace:
        import profhook
        profhook.install()

    res = run_bass_kernel_spmd(nc, in_maps, core_ids=list(range(N_CORES)),
                               trace=trace)
    LAST_RESULTS = res

    logits = np.concatenate([res.results[c]["o_lg"] for c in range(N_CORES)], axis=0)
    cell_states = np.concatenate([res.results[c]["o_cs"] for c in range(N_CORES)],
                                 axis=0)
    rowsums = np.concatenate([
        res.results[c]["o_ms"][:, 0].reshape(MROWS, 8).sum(axis=1)
        for c in range(N_CORES)])
    sparsity_loss = np.float32(rowsums[:M].sum() / (M * G) * 0.01)
    marker_importance = np.concatenate([
        res.results[c]["o_mk"].reshape(CROWS, G) for c in range(N_CORES)], axis=0)
    marker = np.asarray(inputs["marker_weights"], dtype=np.float32)
    marker_scores = np.broadcast_to(marker[None], (B, C, G))
    return (logits, cell_states, sparsity_loss, marker_scores, marker_importance)


# revision 11
# speedup vs baseline: 1.2252x; 1.1666x over previous
"""Trainium2 Bass kernel for the BioHAN model (8-core data-parallel SPMD).

Strategy:
  - Shard the batch dim (B=2048) across 8 NeuronCores (256 rows each);
    replicate all weights.
  - Host pre-transposes each x shard to [G, 256] (gene-major) and appends a
    ones-row so the encoder bias rides inside the big GEMM; the gene dim is
    zero-padded to 10240 and loaded 8 k-tiles per DMA with a
    partition-permuted layout that keeps each partition's read contiguous
    (the gene permutation is irrelevant to the contraction).
  - Encoder GEMM accumulates in PSUM with fp32r (single-pass FP22-truncated
    fp32, 4x the fp32 matmul rate) -> LN -> ReLU -> second encoder GEMM,
    3 attention layers (seq len 1 => attention collapses to
    f @ Wv @ Wo + residual + LN), cosine-sim prototype softmax, classifier.
    Activation transposes run on the PE against an identity matrix.
  - mod_assign softmax row-sums (for sparsity_loss) and marker_weights row
    normalization are sharded across cores AND spread 8 partitions per row
    (DVE cost scales with the free dim, not partitions); cross-partition
    row sums use a block-diagonal ones matmul.
  - Two compiled variants: the "trivial" one skips affine/bias ops that are
    identities for this problem's declared fills (ln gains == 1, biases == 0);
    kernel() inspects the actual inputs and falls back to the general
    variant if they are not.
  - marker_scores is a pure broadcast of marker_weights -> returned as a
    host-side broadcast view (no compute, no copy).
"""
import os
import sys

sys.path.insert(0, '/opt/trn_rl_repo')

import numpy as np

import concourse.bass as bass
import concourse.tile as tile
from concourse import bacc, mybir
from concourse.masks import make_identity

F32 = mybir.dt.float32
F32R = mybir.dt.float32r
AX = mybir.AxisListType.X
AF = mybir.ActivationFunctionType

N_CORES = 8
B, G, HD, H2, C, S, M, L = 2048, 10000, 256, 512, 16, 32, 100, 3
BC = B // N_CORES              # 256 batch rows per core
GP = 10240                     # (G + 1 bias row) zero-padded to 20*512
KT = GP // 128                 # 80 k-tiles in the encoder GEMM
KG = 8                         # k-tiles loaded per DMA
NG = KT // KG                  # 10 DMA groups
MROWS = 13                     # mod_assign rows per core (13*8 >= 100)
CROWS = C // N_CORES           # marker rows per core
EPS = 1e-5
STATE_TEMP = 0.5


def _build_program(trivial):
    nc = bacc.Bacc("TRN2", target_bir_lowering=False, debug=False,
                   num_devices=N_CORES)

    # ---- DRAM I/O ----
    xT = nc.dram_tensor("xT", [GP, BC], F32R, kind="ExternalInput").ap()
    w1a = nc.dram_tensor("w1a", [GP, H2], F32R, kind="ExternalInput").ap()
    w2 = nc.dram_tensor("w2", [H2, HD], F32, kind="ExternalInput").ap()
    wv = nc.dram_tensor("wv", [L, HD, HD], F32, kind="ExternalInput").ap()
    wo = nc.dram_tensor("wo", [L, HD, HD], F32, kind="ExternalInput").ap()
    protos = nc.dram_tensor("protos", [S, HD], F32, kind="ExternalInput").ap()
    cw1 = nc.dram_tensor("cw1", [HD, HD // 2], F32, kind="ExternalInput").ap()
    cw2 = nc.dram_tensor("cw2", [HD // 2, C], F32, kind="ExternalInput").ap()
    bdiag = nc.dram_tensor("bdiag", [128, 128], F32, kind="ExternalInput").ap()
    mod = nc.dram_tensor("mod", [MROWS * 8, G // 8], F32, kind="ExternalInput").ap()
    mkw = nc.dram_tensor("mkw", [CROWS * 8, G // 8], F32, kind="ExternalInput").ap()
    if not trivial:
        eng = nc.dram_tensor("eng", [128, H2], F32, kind="ExternalInput").ap()
        enb = nc.dram_tensor("enb", [128, H2], F32, kind="ExternalInput").ap()
        b2 = nc.dram_tensor("b2", [128, HD], F32, kind="ExternalInput").ap()
        bv = nc.dram_tensor("bv", [128, L, HD], F32, kind="ExternalInput").ap()
        bo = nc.dram_tensor("bo", [128, L, HD], F32, kind="ExternalInput").ap()
        lng = nc.dram_tensor("lng", [128, L, HD], F32, kind="ExternalInput").ap()
        lnb = nc.dram_tensor("lnb", [128, L, HD], F32, kind="ExternalInput").ap()
        cb1 = nc.dram_tensor("cb1", [128, HD // 2], F32, kind="ExternalInput").ap()
        cb2 = nc.dram_tensor("cb2", [128, C], F32, kind="ExternalInput").ap()

    o_cs = nc.dram_tensor("o_cs", [BC, HD], F32, kind="ExternalOutput").ap()
    o_lg = nc.dram_tensor("o_lg", [BC, C], F32, kind="ExternalOutput").ap()
    o_ms = nc.dram_tensor("o_ms", [MROWS * 8, 1], F32, kind="ExternalOutput").ap()
    o_mk = nc.dram_tensor("o_mk", [CROWS * 8, G // 8], F32, kind="ExternalOutput").ap()

    with tile.TileContext(nc) as tc:
        with (
            tc.tile_pool(name="const", bufs=1) as const,
            tc.tile_pool(name="xk", bufs=3) as xkp,
            tc.tile_pool(name="wk", bufs=3) as wkp,
            tc.tile_pool(name="work", bufs=3) as work,
            tc.tile_pool(name="act", bufs=3) as actp,
            tc.tile_pool(name="tposed", bufs=2) as tpp,
            tc.tile_pool(name="small", bufs=4) as sm,
            tc.tile_pool(name="modp", bufs=1) as modp,
            tc.tile_pool(name="acc", bufs=1, space="PSUM") as accp,
            tc.tile_pool(name="tp", bufs=2, space="PSUM") as tpsum,
            tc.tile_pool(name="mm", bufs=3, space="PSUM") as mmpsum,
        ):
            # ---------- constants (gpsimd ring; overlap the encoder) ----------
            ident = const.tile([128, 128], F32, name="ident")
            make_identity(nc, ident)
            eps_t = const.tile([128, 1], F32, name="eps_t")
            nc.vector.memset(eps_t, EPS)

            w2_sb = const.tile([128, H2 // 128, HD], F32, name="w2_sb")
            nc.gpsimd.dma_start(out=w2_sb, in_=w2.rearrange("(t p) n -> p t n", p=128))
            wv_sb = const.tile([128, L, 2, HD], F32, name="wv_sb")
            nc.gpsimd.dma_start(out=wv_sb, in_=wv.rearrange("l (t p) n -> p l t n", p=128))
            wo_sb = const.tile([128, L, 2, HD], F32, name="wo_sb")
            nc.gpsimd.dma_start(out=wo_sb, in_=wo.rearrange("l (t p) n -> p l t n", p=128))
            cw1_sb = const.tile([128, 2, HD // 2], F32, name="cw1_sb")
            nc.gpsimd.dma_start(out=cw1_sb, in_=cw1.rearrange("(t p) n -> p t n", p=128))
            cw2_sb = const.tile([128, C], F32, name="cw2_sb")
            nc.gpsimd.dma_start(out=cw2_sb, in_=cw2)

            if not trivial:
                eng_b = const.tile([128, H2], F32, name="eng_b")
                nc.gpsimd.dma_start(out=eng_b, in_=eng)
                enb_b = const.tile([128, H2], F32, name="enb_b")
                nc.gpsimd.dma_start(out=enb_b, in_=enb)
                b2_b = const.tile([128, HD], F32, name="b2_b")
                nc.gpsimd.dma_start(out=b2_b, in_=b2)
                bv_b = const.tile([128, L, HD], F32, name="bv_b")
                nc.gpsimd.dma_start(out=bv_b, in_=bv)
                bo_b = const.tile([128, L, HD], F32, name="bo_b")
                nc.gpsimd.dma_start(out=bo_b, in_=bo)
                lng_b = const.tile([128, L, HD], F32, name="lng_b")
                nc.gpsimd.dma_start(out=lng_b, in_=lng)
                lnb_b = const.tile([128, L, HD], F32, name="lnb_b")
                nc.gpsimd.dma_start(out=lnb_b, in_=lnb)
                cb1_b = const.tile([128, HD // 2], F32, name="cb1_b")
                nc.gpsimd.dma_start(out=cb1_b, in_=cb1)
                cb2_b = const.tile([128, C], F32, name="cb2_b")
                nc.gpsimd.dma_start(out=cb2_b, in_=cb2)

            # prototypes: raw (zero-padded to 128 partitions) + row-normalized
            # transpose [HD(part-tiles), S]
            pro_sb = const.tile([128, HD], F32, name="pro_sb")
            nc.vector.memset(pro_sb, 0.0)
            nc.gpsimd.dma_start(out=pro_sb[:S, :], in_=protos)
            pn_sq = sm.tile([S, HD], F32, name="pn_sq")
            nc.vector.tensor_mul(pn_sq, pro_sb[:S, :], pro_sb[:S, :])
            pn = sm.tile([S, 1], F32, name="pn")
            nc.vector.reduce_sum(out=pn, in_=pn_sq, axis=AX)
            nc.scalar.activation(out=pn, in_=pn, func=AF.Sqrt)
            rpn = sm.tile([S, 1], F32, name="rpn")
            nc.vector.reciprocal(out=rpn, in_=pn)
            pro_n = const.tile([S, HD], F32, name="pro_n")
            nc.vector.tensor_scalar_mul(out=pro_n, in0=pro_sb[:S, :], scalar1=rpn)
            proT = const.tile([128, 2, S], F32, name="proT")
            for j in range(2):
                pt = tpsum.tile([128, 128], F32, name="pt_pro", tag="tps")
                nc.tensor.transpose(pt[:, :S], pro_n[:, j * 128:(j + 1) * 128],
                                    ident[:S, :S])
                nc.vector.tensor_copy(out=proT[:, j, :], in_=pt[:, :S])

            # ---------- mod_assign softmax row-sums (rows spread 8-wide) ----------
            MP = MROWS * 8
            bd_sb = const.tile([128, 128], F32, name="bd_sb")
            nc.gpsimd.dma_start(out=bd_sb, in_=bdiag)
            msh = modp.tile([MP, G // 8], F32, name="msh")
            nc.gpsimd.dma_start(out=msh, in_=mod)
            nc.scalar.activation(out=msh, in_=msh, func=AF.Exp)
            mcs = sm.tile([MP, 1], F32, name="mcs")
            nc.vector.reduce_sum(out=mcs, in_=msh, axis=AX)
            mden_ps = tpsum.tile([128, 128], F32, name="mden_ps", tag="tps")
            nc.tensor.matmul(mden_ps[:MP, :1], bd_sb[:MP, :MP], mcs,
                             start=True, stop=True)
            rmden = sm.tile([MP, 1], F32, name="rmden")
            nc.vector.reciprocal(out=rmden, in_=mden_ps[:MP, :1])
            nc.vector.tensor_scalar_mul(out=msh, in0=msh, scalar1=rmden)
            msum2 = sm.tile([MP, 1], F32, name="msum2")
            nc.vector.reduce_sum(out=msum2, in_=msh, axis=AX)
            nc.sync.dma_start(out=o_ms, in_=msum2)

            # ---------- marker importance (rows spread 8-wide) ----------
            CP = CROWS * 8
            mk = modp.tile([CP, G // 8], F32, name="mk")
            nc.gpsimd.dma_start(out=mk, in_=mkw)
            mkcs = sm.tile([CP, 1], F32, name="mkcs")
            nc.vector.reduce_sum(out=mkcs, in_=mk, axis=AX,
                                 apply_absolute_value=True)
            mkden_ps = tpsum.tile([128, 128], F32, name="mkden_ps", tag="tps")
            nc.tensor.matmul(mkden_ps[:CP, :1], bd_sb[:CP, :CP], mkcs,
                             start=True, stop=True)
            mkden = sm.tile([CP, 1], F32, name="mkden")
            nc.vector.tensor_scalar_max(out=mkden, in0=mkden_ps[:CP, :1],
                                        scalar1=1e-12)
            rmks = sm.tile([CP, 1], F32, name="rmks")
            nc.vector.reciprocal(out=rmks, in_=mkden)
            nc.vector.tensor_scalar_mul(out=mk, in0=mk, scalar1=rmks)
            nc.sync.dma_start(out=o_mk, in_=mk)

            # ---------- encoder GEMM: y1[BC, H2] = xT.T @ w1a ----------
            acc = [accp.tile([128, H2], F32, name=f"acc{m}") for m in range(2)]
            xTg = xT.rearrange("(g p t) b -> g p t b", p=128, t=KG)
            w1g = w1a.rearrange("(g p t) n -> g p t n", p=128, t=KG)
            for g in range(NG):
                xk = xkp.tile([128, KG, BC], F32R, name="xk")
                nc.scalar.dma_start(out=xk, in_=xTg[g])
                wk = wkp.tile([128, KG, H2], F32R, name="wk")
                nc.sync.dma_start(out=wk, in_=w1g[g])
                for t in range(KG):
                    for m in range(2):
                        nc.tensor.matmul(acc[m], xk[:, t, m * 128:(m + 1) * 128],
                                         wk[:, t, :],
                                         start=(g == 0 and t == 0),
                                         stop=(g == NG - 1 and t == KG - 1))

            # ---------- helpers ----------
            def layer_norm(x_in, out_sb, g_ap, b_ap):
                stats = sm.tile([128, 6], F32, name="lnstats")
                nc.vector.bn_stats(out=stats, in_=x_in)
                mv = sm.tile([128, 2], F32, name="lnmv")
                nc.vector.bn_aggr(out=mv, in_=stats)
                std = sm.tile([128, 1], F32, name="lnstd")
                nc.scalar.activation(out=std, in_=mv[:, 1:2], func=AF.Sqrt,
                                     bias=eps_t)
                rstd = sm.tile([128, 1], F32, name="lnrstd")
                nc.vector.reciprocal(out=rstd, in_=std)
                nc.vector.tensor_scalar(out=out_sb, in0=x_in,
                                        scalar1=mv[:, 0:1], scalar2=rstd,
                                        op0=mybir.AluOpType.subtract,
                                        op1=mybir.AluOpType.mult)
                if not trivial:
                    nc.vector.tensor_mul(out_sb, out_sb, g_ap)
                    nc.vector.tensor_add(out_sb, out_sb, b_ap)

            def transpose_to(src_sb, n_blocks, name):
                """src [128, n_blocks*128] -> SBUF [128, n_blocks, 128],
                transposing 128x128 blocks pairwise into one PSUM bank so a
                single DVE copy evicts two blocks."""
                dst = tpp.tile([128, n_blocks, 128], F32, name=name)
                j = 0
                while j < n_blocks:
                    w = min(2, n_blocks - j)
                    pt = tpsum.tile([128, 256], F32, name="pt_" + name, tag="tps")
                    for i in range(w):
                        nc.tensor.transpose(pt[:, i * 128:(i + 1) * 128],
                                            src_sb[:, (j + i) * 128:(j + i + 1) * 128],
                                            ident)
                    nc.vector.tensor_copy(out=dst[:, j:j + w, :],
                                          in_=pt[:, :w * 128])
                    j += w
                return dst

            def evict(ps, out_name, bias_ap, pool=actp):
                """PSUM -> SBUF, adding bias_ap unless trivial."""
                t = pool.tile([128, ps.shape[-1]], F32, name=out_name)
                if trivial or bias_ap is None:
                    nc.vector.tensor_copy(out=t, in_=ps)
                else:
                    nc.vector.tensor_add(t, ps, bias_ap)
                return t

            # ---------- per m-tile tail ----------
            for m in range(2):
                rows = slice(m * 128, (m + 1) * 128)
                h = work.tile([128, H2], F32, name="h")
                layer_norm(acc[m], h, eng_b if not trivial else None,
                           enb_b if not trivial else None)
                nc.scalar.activation(out=h, in_=h, func=AF.Relu)

                # gene_features = h @ w2 + b2
                hT = transpose_to(h, 4, "hT")
                gf_ps = mmpsum.tile([128, HD], F32, name="gf_ps", tag="mmps")
                for j in range(4):
                    nc.tensor.matmul(gf_ps, hT[:, j, :], w2_sb[:, j, :],
                                     start=(j == 0), stop=(j == 3))
                f = evict(gf_ps, "f", None if trivial else b2_b)

                # attention layers (seq len 1): f = LN(f @ Wv @ Wo + biases + f)
                for l in range(L):
                    fT = transpose_to(f, 2, "fT")
                    o1_ps = mmpsum.tile([128, HD], F32, name="o1_ps", tag="mmps")
                    for j in range(2):
                        nc.tensor.matmul(o1_ps, fT[:, j, :], wv_sb[:, l, j, :],
                                         start=(j == 0), stop=(j == 1))
                    o1 = evict(o1_ps, "o1", None if trivial else bv_b[:, l, :])
                    o1T = transpose_to(o1, 2, "o1T")
                    o2_ps = mmpsum.tile([128, HD], F32, name="o2_ps", tag="mmps")
                    for j in range(2):
                        nc.tensor.matmul(o2_ps, o1T[:, j, :], wo_sb[:, l, j, :],
                                         start=(j == 0), stop=(j == 1))
                    o2 = actp.tile([128, HD], F32, name="o2")
                    nc.vector.tensor_add(o2, o2_ps, f)          # residual
                    if not trivial:
                        nc.vector.tensor_add(o2, o2, bo_b[:, l, :])
                    f = actp.tile([128, HD], F32, name="f")
                    layer_norm(o2, f, lng_b[:, l, :] if not trivial else None,
                               lnb_b[:, l, :] if not trivial else None)

                # cosine-sim prototype softmax
                fsq = work.tile([128, HD], F32, name="fsq")
                nc.vector.tensor_mul(fsq, f, f)
                fn = sm.tile([128, 1], F32, name="fn")
                nc.vector.reduce_sum(out=fn, in_=fsq, axis=AX)
                nc.scalar.activation(out=fn, in_=fn, func=AF.Sqrt)
                rfn = sm.tile([128, 1], F32, name="rfn")
                nc.vector.reciprocal(out=rfn, in_=fn)
                rfn2 = sm.tile([128, 1], F32, name="rfn2")
                nc.vector.tensor_scalar_mul(out=rfn2, in0=rfn,
                                            scalar1=1.0 / STATE_TEMP)
                fT = transpose_to(f, 2, "fT2")
                sim_ps = mmpsum.tile([128, HD], F32, name="sim_ps", tag="mmps")[:, :S]
                for j in range(2):
                    nc.tensor.matmul(sim_ps, fT[:, j, :], proT[:, j, :],
                                     start=(j == 0), stop=(j == 1))
                sw = work.tile([128, S], F32, name="sw")
                nc.vector.tensor_scalar_mul(out=sw, in0=sim_ps, scalar1=rfn2)
                smx = sm.tile([128, 1], F32, name="smx")
                nc.vector.reduce_max(out=smx, in_=sw, axis=AX)
                nsmx = sm.tile([128, 1], F32, name="nsmx")
                nc.vector.tensor_scalar_mul(out=nsmx, in0=smx, scalar1=-1.0)
                nc.scalar.activation(out=sw, in_=sw, func=AF.Exp, bias=nsmx)
                ssum = sm.tile([128, 1], F32, name="ssum")
                nc.vector.reduce_sum(out=ssum, in_=sw, axis=AX)
                rssum = sm.tile([128, 1], F32, name="rssum")
                nc.vector.reciprocal(out=rssum, in_=ssum)
                nc.vector.tensor_scalar_mul(out=sw, in0=sw, scalar1=rssum)

                # cell_states = sw @ prototypes  (pad K=32 -> 128 with zeros)
                swT = tpp.tile([128, 128], F32, name="swT")
                nc.vector.memset(swT, 0.0)
                swT_ps = tpsum.tile([128, 256], F32, name="swT_ps", tag="tps")
                nc.tensor.transpose(swT_ps[:S, :128], sw, ident)
                nc.vector.tensor_copy(out=swT[:S, :], in_=swT_ps[:S, :128])
                cs_ps = mmpsum.tile([128, HD], F32, name="cs_ps", tag="mmps")
                nc.tensor.matmul(cs_ps, swT, pro_sb, start=True, stop=True)
                cs = evict(cs_ps, "cs", None)
                nc.sync.dma_start(out=o_cs[rows, :], in_=cs)

                # classifier head
                csT = transpose_to(cs, 2, "csT")
                h1_ps = mmpsum.tile([128, HD], F32, name="h1_ps", tag="mmps")[:, :HD // 2]
                for j in range(2):
                    nc.tensor.matmul(h1_ps, csT[:, j, :], cw1_sb[:, j, :],
                                     start=(j == 0), stop=(j == 1))
                h1 = work.tile([128, HD // 2], F32, name="h1")
                if trivial:
                    nc.scalar.activation(out=h1, in_=h1_ps, func=AF.Relu)
                else:
                    nc.vector.tensor_add(h1, h1_ps, cb1_b)
                    nc.scalar.activation(out=h1, in_=h1, func=AF.Relu)
                h1T = transpose_to(h1, 1, "h1T")
                lg_ps = mmpsum.tile([128, HD], F32, name="lg_ps", tag="mmps")[:, :C]
                nc.tensor.matmul(lg_ps, h1T[:, 0, :], cw2_sb, start=True, stop=True)
                lg = evict(lg_ps, "lg", None if trivial else cb2_b, pool=work)
                nc.sync.dma_start(out=o_lg[rows, :], in_=lg)

    nc.finalize()
    return nc


_CACHE = {}


def get_program(trivial=False):
    key = bool(trivial)
    if key not in _CACHE:
        _CACHE[key] = _build_program(key)
    return _CACHE[key]


def _is_trivial(inputs):
    qkv_b = np.asarray(inputs["qkv_b"])
    checks = [
        (inputs["enc_ln_g"], 1.0), (inputs["enc_ln_b"], 0.0),
        (inputs["enc_b2"], 0.0), (qkv_b[:, 2 * HD:3 * HD], 0.0),
        (inputs["out_b"], 0.0), (inputs["ln_g"], 1.0), (inputs["ln_b"], 0.0),
        (inputs["clf_b1"], 0.0), (inputs["clf_b2"], 0.0),
    ]
    return all(np.all(np.asarray(v) == c) for v, c in checks)


def make_in_maps(inputs, trivial):
    """Build the 8 per-core input dicts from the full-size input dict."""
    x = np.asarray(inputs["x"], dtype=np.float32)
    w1 = np.asarray(inputs["enc_w1"], dtype=np.float32)
    w1a = np.zeros((GP, H2), dtype=np.float32)
    w1a[:G] = w1
    w1a[G] = np.asarray(inputs["enc_b1"], dtype=np.float32)

    qkv_w = np.asarray(inputs["qkv_w"], dtype=np.float32)
    qkv_b = np.asarray(inputs["qkv_b"], dtype=np.float32)

    def rep(v):
        v = np.asarray(v, dtype=np.float32)
        return np.ascontiguousarray(np.broadcast_to(v[None], (128,) + v.shape))

    common = {
        "w1a": w1a,
        "w2": np.asarray(inputs["enc_w2"], dtype=np.float32),
        "wv": np.ascontiguousarray(qkv_w[:, :, 2 * HD:3 * HD]),
        "wo": np.asarray(inputs["out_w"], dtype=np.float32),
        "protos": np.asarray(inputs["prototypes"], dtype=np.float32),
        "cw1": np.asarray(inputs["clf_w1"], dtype=np.float32),
        "cw2": np.asarray(inputs["clf_w2"], dtype=np.float32),
    }
    if not trivial:
        common.update({
            "eng": rep(inputs["enc_ln_g"]),
            "enb": rep(inputs["enc_ln_b"]),
            "b2": rep(inputs["enc_b2"]),
            "bv": rep(qkv_b[:, 2 * HD:3 * HD]),
            "bo": rep(inputs["out_b"]),
            "lng": rep(inputs["ln_g"]),
            "lnb": rep(inputs["ln_b"]),
            "cb1": rep(inputs["clf_b1"]),
            "cb2": rep(inputs["clf_b2"]),
        })
    mod_assign = np.asarray(inputs["mod_assign"], dtype=np.float32)
    marker = np.asarray(inputs["marker_weights"], dtype=np.float32)
    bdiag = np.kron(np.eye(16, dtype=np.float32), np.ones((8, 8), np.float32))

    in_maps = []
    for c in range(N_CORES):
        xs = x[c * BC:(c + 1) * BC, :]          # [BC, G]
        xTc = np.zeros((GP, BC), dtype=np.float32)
        xTc[:G] = xs.T
        xTc[G] = 1.0
        mrows = np.minimum(np.arange(c * MROWS, (c + 1) * MROWS), M - 1)
        in_maps.append({
            "xT": np.ascontiguousarray(xTc),
            "mod": np.ascontiguousarray(
                mod_assign[mrows].reshape(MROWS * 8, G // 8)),
            "mkw": np.ascontiguousarray(
                marker[c * CROWS:(c + 1) * CROWS].reshape(CROWS * 8, G // 8)),
            "bdiag": bdiag,
            **common,
        })
    return in_maps


LAST_RESULTS = None


def kernel(**inputs):
    global LAST_RESULTS
    from concourse.bass_utils import run_bass_kernel_spmd

    trivial = _is_trivial(inputs)
    nc = get_program(trivial)
    in_maps = make_in_maps(inputs, trivial)

    trace = bool(int(os.environ.get("KERNEL_TRACE", "0")))
    if trace:
        import profhook
        profhook.install()

    res = run_bass_kernel_spmd(nc, in_maps, core_ids=list(range(N_CORES)),
                               trace=trace)
    LAST_RESULTS = res

    logits = np.concatenate([res.results[c]["o_lg"] for c in range(N_CORES)], axis=0)
    cell_states = np.concatenate([res.results[c]["o_cs"] for c in range(N_CORES)],
                                 axis=0)
    rowsums = np.concatenate([
        res.results[c]["o_ms"][:, 0].reshape(MROWS, 8).sum(axis=1)
        for c in range(N_CORES)])
    sparsity_loss = np.float32(rowsums[:M].sum() / (M * G) * 0.01)
    marker_importance = np.concatenate([
        res.results[c]["o_mk"].reshape(CROWS, G) for c in range(N_CORES)], axis=0)
    marker = np.asarray(inputs["marker_weights"], dtype=np.float32)
    marker_scores = np.broadcast_to(marker[None], (B, C, G))
    return (logits, cell_states, sparsity_loss, marker_scores, marker_importance)
